# revision 1
# baseline (speedup 1.0000x reference)
"""ClusterKLLoss Trainium2 kernel (8 NeuronCores, data-parallel over rows of c_i).

Math (derived from the reference):
  loss = CE(logits, arange(B), sum) / B  with logits[i,j] = -kl[i,j]/T
  kl[i,j] = hneg[j] - Li[i] . Q[j],  Q = softmax(c_j), hneg[j] = sum Q log Q.
  Per-row (i) constant shifts cancel in log-softmax, so log_softmax(c_i) is
  never needed:
    G[i,j] = (c_i[i] . Q[j] - hneg[j]) / T       (logits up to per-row shift)
  With E = exp(c_j) (no max-sub needed for N(0,1) inputs), Z_j = sum_k E[j,k],
  A_j = sum_k E[j,k] c_j[j,k]:
    hneg_j = A_j/Z_j - ln Z_j
    G[i,j] = (S[i,j] + e_j) * s_j,  S = c_i @ E^T,  e_j = Z_j ln Z_j - A_j,
    s_j = 1/(T Z_j)
  loss = sum_i (logsumexp_j G[i,j] - G[i,i]) / B

Sharding: core c takes c_i rows [512c, 512c+512) and a rotated copy of c_j
(np.roll(c_j, -512c, axis=0)) so the diagonal lands at local columns
[0, 512) on every core -> one SPMD NEFF, no per-core addressing. Row
logsumexp is permutation-invariant so the rotation changes nothing else.
Each core returns its scalar partial; the host sums 8 partials / B.
"""

import sys

for _p in ("/opt/trn_rl_repo",):
    if _p not in sys.path:
        sys.path.insert(0, _p)

import numpy as np

import concourse.bass as bass
import concourse.bacc as bacc
import concourse.tile as tile
from concourse import mybir
from concourse import bass_utils

B = 4096
D = 2048
TEMP = 0.5
NCORES = 8
SHARD = B // NCORES  # 512
KT = D // 128  # 16 k partition-tiles
NCH = 8  # 512-wide column chunks
F32 = mybir.dt.float32
F16 = mybir.dt.float16
AF = mybir.ActivationFunctionType
OP = mybir.AluOpType
AX = mybir.AxisListType

NEG_INF = -3.0e38

import os
LOADS_GPSIMD = os.environ.get("K_LOADS_GPSIMD", "0") == "1"
XPOSE_SCALAR = os.environ.get("K_XPOSE_SCALAR", "0") == "1"
PROD_MOD = int(os.environ.get("K_PROD_MOD", "4"))


CSCALE = 4096.0  # power-of-two normalizer keeping W' = E*s*C in fp16 normal range
INV_C = 1.0 / CSCALE


def build_kernel_body(tc, out_ap, ci_ap, cj_ap, eye_ap, reps=1):
    """Emit the kernel IR. out: [1,1] f32; ci: [512,2048] f32;
    cj: [4096,2048] f32 (rotated per-core); eye: [128,128] f32.

    v2: per-j softmax scale is folded into the fp16 rhs operand
    (W' = E * C/(T*Z_j)), the bias row rides the matmul as two fp16 hi/lo
    K-rows, and the row-softmax needs no max subtraction (G in [-11, 27]),
    so ACT consumes PSUM directly: exp(S*2^-12) with free row-sum accum.
    """
    nc = tc.nc

    from contextlib import ExitStack

    with ExitStack() as ctx:
        singles = ctx.enter_context(tc.tile_pool(name="singles", bufs=1))
        xpool = ctx.enter_context(tc.tile_pool(name="xpool", bufs=3))
        epool = ctx.enter_context(tc.tile_pool(name="epool", bufs=3))
        etpool = ctx.enter_context(tc.tile_pool(name="etpool", bufs=5))
        spool = ctx.enter_context(tc.tile_pool(name="spool", bufs=3))
        psS = ctx.enter_context(tc.tile_pool(name="psS", bufs=6, space="PSUM"))
        psX = ctx.enter_context(tc.tile_pool(name="psX", bufs=2, space="PSUM"))

        # constants
        eye32 = singles.tile([128, 128], F32)
        nc.sync.dma_start(out=eye32, in_=eye_ap)
        eye16 = singles.tile([128, 128], F16)
        nc.vector.tensor_copy(out=eye16, in_=eye32)
        ones2 = singles.tile([2, 128], F16)
        nc.vector.memset(ones2, 1.0)
        onesc = singles.tile([128, 1], F32)
        nc.vector.memset(onesc, 1.0)

        # per-j scalar accumulators (col t = j-tile t)
        Zc = singles.tile([128, 32], F32)
        Ac = singles.tile([128, 32], F32)
        sCc = singles.tile([128, 32], F32)
        Zparts = singles.tile([128, 32], F32)  # col = m*8 + n
        Dc = singles.tile([128, 4], F32)
        Zi = singles.tile([128, 4], F32)

        # ci -> fp16 -> transposed [k-part, i] layout
        ciT = singles.tile([128, 4, KT, 128], F16)
        for t in range(4):
            cit = xpool.tile([128, D], F32, tag="xload")
            nc.sync.dma_start(out=cit, in_=ci_ap[128 * t : 128 * (t + 1), :])
            c16 = epool.tile([128, D], F16, tag="estg")
            nc.vector.tensor_copy(out=c16, in_=cit)
            nc.sync.dma_start_transpose(out=ciT[:, t], in_=c16)

        for _rep in range(reps):
            _run_main(tc, ctx, out_ap, cj_ap, locals())


def _run_main(tc, ctx, out_ap, cj_ap, env):
    nc = tc.nc
    singles = env["singles"]; xpool = env["xpool"]; epool = env["epool"]
    etpool = env["etpool"]; spool = env["spool"]; psS = env["psS"]; psX = env["psX"]
    eye32 = env["eye32"]; eye16 = env["eye16"]; ones2 = env["ones2"]; onesc = env["onesc"]
    Zc = env["Zc"]; Ac = env["Ac"]; sCc = env["sCc"]; Zparts = env["Zparts"]
    Dc = env["Dc"]; Zi = env["Zi"]; ciT = env["ciT"]
    if True:
        for n in range(NCH):
            ETc = etpool.tile([128, 4, KT, 128], F16, tag="et")
            for q in range(4):
                t = 4 * n + q
                xt = xpool.tile([128, D], F32, tag="xload")
                (nc.gpsimd if LOADS_GPSIMD else nc.sync).dma_start(
                    out=xt, in_=cj_ap[128 * t : 128 * (t + 1), :]
                )
                es = epool.tile([128, D], F16, tag="estg")
                # E = exp(x); Z_j accumulated for free
                nc.scalar.activation(
                    out=es, in_=xt, func=AF.Exp, accum_out=Zc[:, t : t + 1]
                )
                # A_j = sum_k E*x: product (split DVE/GPSIMD) + DVE reduce
                prod = epool.tile([128, D], F16, tag="prod")
                if t % PROD_MOD == 0:
                    nc.vector.tensor_mul(prod, es, xt)
                else:
                    nc.gpsimd.tensor_mul(prod, es, xt)
                nc.vector.tensor_reduce(
                    out=Ac[:, t : t + 1], in_=prod, axis=AX.X, op=OP.add
                )
                # sC_j = C/(T*Z_j); W' = E*sC in fp16 (normal range)
                nc.vector.tensor_scalar_mul(
                    sCc[:, t : t + 1], Zc[:, t : t + 1], float(TEMP / CSCALE)
                )
                nc.vector.reciprocal(
                    out=sCc[:, t : t + 1], in_=sCc[:, t : t + 1]
                )
                ws = epool.tile([128, D], F16, tag="ws")
                nc.vector.tensor_scalar_mul(ws, es, sCc[:, t : t + 1])
                # W'^T into this chunk's rhs tile (contiguous 3D dest)
                (nc.scalar if XPOSE_SCALAR else nc.sync).dma_start_transpose(
                    out=ETc[:, q], in_=ws
                )

            # per-chunk bias row: b' = (lnZ - A/Z)*(C/T) = lnZ*(C/T) - A*sC
            z4 = Zc[:, 4 * n : 4 * n + 4]
            a4 = Ac[:, 4 * n : 4 * n + 4]
            lnz = spool.tile([128, 4], F32, tag="lnz")
            nc.scalar.activation(out=lnz, in_=z4, func=AF.Ln)
            bp = spool.tile([128, 4], F32, tag="bp")
            nc.vector.tensor_mul(bp, a4, sCc[:, 4 * n : 4 * n + 4])
            lnzs = spool.tile([128, 4], F32, tag="lnzs")
            nc.vector.tensor_scalar_mul(lnzs, lnz, float(CSCALE / TEMP))
            nc.vector.tensor_sub(bp, lnzs, bp)
            # split bias into fp16 hi+lo (keeps fp32 accuracy in the matmul)
            e2 = spool.tile([128, 4, 2], F16, tag="e2")
            nc.vector.tensor_copy(out=e2[:, :, 0], in_=bp)
            nc.vector.tensor_sub(e2[:, :, 1], bp, e2[:, :, 0])
            # transpose per q and collect into one [2, 512] rhs row pair
            e2row = spool.tile([2, 512], F16, tag="e2row")
            for q in range(4):
                e2q_ps = psX.tile([2, 128], F16, tag="xp", bufs=1, name=f"e2ps{n}_{q}")
                nc.tensor.transpose(e2q_ps, e2[:, q, :], eye16)
                nc.vector.tensor_copy(
                    out=e2row[:, 128 * q : 128 * (q + 1)], in_=e2q_ps
                )

            # main matmuls; ACT consumes PSUM directly (exp + row-sum accum)
            for m in range(4):
                S_ps = psS.tile([128, 512], F32, tag="s")
                for kt in range(KT):
                    nc.tensor.matmul(
                        S_ps,
                        ciT[:, m, kt, :],
                        ETc[:, :, kt, :],
                        start=(kt == 0),
                        stop=False,
                    )
                nc.tensor.matmul(S_ps, ones2, e2row, start=False, stop=True)
                if n == 0:
                    junk = spool.tile([128, 128], F32, tag="junk")
                    nc.vector.tensor_mul(
                        junk, S_ps[:, 128 * m : 128 * (m + 1)], eye32
                    )
                    nc.vector.tensor_reduce(
                        out=Dc[:, m : m + 1], in_=junk, axis=AX.X, op=OP.add
                    )
                expj = spool.tile([128, 512], F16, tag="expj", bufs=2)
                nc.scalar.activation(
                    out=expj,
                    in_=S_ps,
                    func=AF.Exp,
                    scale=float(INV_C),
                    accum_out=Zparts[:, 8 * m + n : 8 * m + n + 1],
                )

        # lse_i = ln(sum_n Zparts); loss terms = lse - diag*2^-12
        Zp = Zparts.rearrange("p (m n) -> p m n", n=8)
        nc.vector.tensor_reduce(out=Zi, in_=Zp, axis=AX.X, op=OP.add)
        lnzi = spool.tile([128, 4], F32, tag="lnzi")
        nc.scalar.activation(out=lnzi, in_=Zi, func=AF.Ln)
        gd = spool.tile([128, 4], F32, tag="gd")
        nc.vector.tensor_scalar_mul(gd, Dc, float(INV_C))
        terms = spool.tile([128, 4], F32, tag="terms")
        nc.vector.tensor_sub(terms, lnzi, gd)
        part_ps = psX.tile([1, 4], F32, tag="xp", bufs=1)
        nc.tensor.matmul(part_ps, onesc, terms, start=True, stop=True)
        part = spool.tile([1, 4], F32, tag="part")
        nc.vector.tensor_copy(out=part, in_=part_ps)
        res = spool.tile([1, 1], F32, tag="res")
        nc.vector.reduce_sum(out=res, in_=part, axis=AX.X)
        nc.sync.dma_start(out=out_ap, in_=res)


_NC_CACHE = {}


def build_nc(reps=1):
    key = ("nc", reps)
    if key in _NC_CACHE:
        return _NC_CACHE[key]
    nc = bacc.Bacc("TRN2", target_bir_lowering=False, debug=False)
    ci = nc.dram_tensor("ci", [SHARD, D], F32, kind="ExternalInput").ap()
    cj = nc.dram_tensor("cj", [B, D], F32, kind="ExternalInput").ap()
    eye = nc.dram_tensor("eye", [128, 128], F32, kind="ExternalInput").ap()
    out = nc.dram_tensor("out", [1, 1], F32, kind="ExternalOutput").ap()
    with tile.TileContext(nc) as tc:
        build_kernel_body(tc, out, ci, cj, eye, reps=reps)
    nc.compile()
    _NC_CACHE[key] = nc
    return nc


def make_in_maps(c_i, c_j):
    eye = np.eye(128, dtype=np.float32)
    in_maps = []
    for c in range(NCORES):
        in_maps.append(
            {
                "ci": np.ascontiguousarray(c_i[SHARD * c : SHARD * (c + 1)]),
                "cj": np.ascontiguousarray(np.roll(c_j, -SHARD * c, axis=0)),
                "eye": eye,
            }
        )
    return in_maps


def kernel(c_i, c_j, **kwargs):
    c_i = np.ascontiguousarray(np.asarray(c_i, dtype=np.float32))
    c_j = np.ascontiguousarray(np.asarray(c_j, dtype=np.float32))
    nc = build_nc()
    in_maps = make_in_maps(c_i, c_j)
    res = bass_utils.run_bass_kernel_spmd(
        nc, in_maps, core_ids=list(range(NCORES))
    )
    total = np.float64(0.0)
    for r in res.results:
        total += np.float64(r["out"][0, 0])
    return np.float32(total / B).reshape(())



# revision 8
# speedup vs baseline: 1.0952x; 1.0952x over previous
"""ClusterKLLoss Trainium2 kernel (8 NeuronCores, 2D-sharded: 2 i-halves x 4
j-quarters).

Math (derived from the reference):
  loss = CE(logits, arange(B), sum) / B  with logits[i,j] = -kl[i,j]/T
  kl[i,j] = hneg[j] - Li[i] . Q[j],  Q = softmax(c_j), hneg[j] = sum Q log Q.
  Per-row (i) constant shifts cancel in log-softmax, so log_softmax(c_i) is
  never needed:
    G[i,j] = (c_i[i] . Q[j] - hneg[j]) / T       (logits up to per-row shift)
  With E = exp(c_j), Z_j = sum E[j], A_j = sum E[j]*c_j[j]:
    S[i,j] = c_i[i] . W'[j] + b'_j = G[i,j]*C,   W' = E*C/(T*Z_j),
    b'_j = (ln Z_j)*(C/T) - A_j*(C/(T*Z_j))
  loss = sum_i (ln sum_j exp(S[i,j]/C) - S[i,i]/C) / B

Sharding: core c = (h, p), h = c//4, p = c%4. Core holds c_i rows
[2048h, 2048h+2048) and c_j rows [2048h+1024p, +1024) mod B. The four cores
of a half together cover all j, so the host sums their per-row partial
softmax sums and takes the log. The diagonal S[i,i] lands in cores p=0
(m-tiles 0-7) and p=1 (m-tiles 8-15); every core extracts the same local
window (cols 128*(m%8)) and the host keeps the valid ones.

Per-core outputs: out[:, 0:16] = Zi (partial sum_j exp(S/C) per i, one col
per 128-row m-tile), out[:, 16:32] = Dc (local diag of S).
"""

import sys

for _p in ("/opt/trn_rl_repo",):
    if _p not in sys.path:
        sys.path.insert(0, _p)

import numpy as np

import concourse.bass as bass
import concourse.bacc as bacc
import concourse.tile as tile
from concourse import mybir
from concourse import bass_utils

B = 4096
D = 2048
TEMP = 0.5
NCORES = 8
IH = 2      # i halves
JQ = 4      # j quarters
ISH = B // IH   # 2048 rows of c_i per core
JSH = B // JQ   # 1024 rows of c_j per core
MT = ISH // 128  # 16 i-tiles
JT = JSH // 128  # 8 j-tiles
KT = D // 128    # 16 k partition-tiles
F32 = mybir.dt.float32
F16 = mybir.dt.float16
AF = mybir.ActivationFunctionType
OP = mybir.AluOpType
AX = mybir.AxisListType

CSCALE = 4096.0  # power-of-two normalizer keeping W' = E*C/(T*Z) in fp16 range
INV_C = 1.0 / CSCALE


def build_kernel_body(tc, out_ap, ci_ap, cj_ap, eye_ap):
    """out: [128,32] f32; ci: [2048,2048] f32; cj: [1024,2048] f32;
    eye: [128,128] f32."""
    nc = tc.nc

    from contextlib import ExitStack

    with ExitStack() as ctx:
        singles = ctx.enter_context(tc.tile_pool(name="singles", bufs=1))
        xpool = ctx.enter_context(tc.tile_pool(name="xpool", bufs=3))
        epool = ctx.enter_context(tc.tile_pool(name="epool", bufs=3))
        spool = ctx.enter_context(tc.tile_pool(name="spool", bufs=3))
        dpool = ctx.enter_context(tc.tile_pool(name="dpool", bufs=2))
        psS = ctx.enter_context(tc.tile_pool(name="psS", bufs=2, space="PSUM"))
        psX = ctx.enter_context(tc.tile_pool(name="psX", bufs=2, space="PSUM"))

        # constants
        eye32 = singles.tile([128, 128], F32)
        nc.sync.dma_start(out=eye32, in_=eye_ap)
        eye16 = singles.tile([128, 128], F16)
        nc.vector.tensor_copy(out=eye16, in_=eye32)
        ones2 = singles.tile([2, 128], F16)
        nc.vector.memset(ones2, 1.0)

        # per-j scalars, one col per local cj tile
        Zc = singles.tile([128, JT], F32)
        Asc = singles.tile([128, JT], F32)   # A_j * sC_j
        sCc = singles.tile([128, JT], F32)
        # outputs
        Zi = singles.tile([128, 2 * MT], F32)
        Dc = singles.tile([128, MT], F32)
        # operand stores
        WT = singles.tile([128, KT, JT, 128], F16)   # [k, kt, jt, j]
        ciT = singles.tile([128, MT, KT, 128], F16)  # [k, m, kt, i]
        biasr = singles.tile([2, JSH], F16)          # bias hi/lo rows

        # ---- c_j preprocessing: 8 tiles -> WT + per-j scalars ----
        for q in range(JT):
            xt = xpool.tile([128, D], F32, tag="xload")
            nc.sync.dma_start(out=xt, in_=cj_ap[128 * q : 128 * (q + 1), :])
            es = epool.tile([128, D], F16, tag="es")
            # E = exp(x); Z_j accumulated for free
            nc.scalar.activation(
                out=es, in_=xt, func=AF.Exp, accum_out=Zc[:, q : q + 1]
            )
            # sC_j = C/(T*Z_j)
            nc.vector.tensor_scalar_mul(
                sCc[:, q : q + 1], Zc[:, q : q + 1], float(TEMP / CSCALE)
            )
            nc.vector.reciprocal(out=sCc[:, q : q + 1], in_=sCc[:, q : q + 1])
            # A_j*sC_j = sum_k (E*sC)*x ; W' = E*sC in fp16
            ws = epool.tile([128, D], F16, tag="ws")
            nc.vector.tensor_scalar_mul(ws, es, sCc[:, q : q + 1])
            prod = epool.tile([128, D], F16, tag="prod")
            nc.vector.tensor_mul(prod, ws, xt)
            nc.vector.tensor_reduce(
                out=Asc[:, q : q + 1], in_=prod, axis=AX.X, op=OP.add
            )
            # W'^T into the rhs store (dest strided over kt)
            nc.scalar.dma_start_transpose(out=WT[:, :, q, :], in_=ws)

        # ---- bias row: b' = lnZ*(C/T) - A*sC, split into fp16 hi+lo ----
        lnz = spool.tile([128, JT], F32, tag="lnz")
        nc.scalar.activation(out=lnz, in_=Zc, func=AF.Ln)
        bp = spool.tile([128, JT], F32, tag="bp")
        nc.vector.tensor_scalar_mul(bp, lnz, float(CSCALE / TEMP))
        nc.vector.tensor_sub(bp, bp, Asc)
        e2 = spool.tile([128, JT, 2], F16, tag="e2")
        nc.vector.tensor_copy(out=e2[:, :, 0], in_=bp)
        nc.vector.tensor_sub(e2[:, :, 1], bp, e2[:, :, 0])
        for q in range(JT):
            e2q_ps = psX.tile([2, 128], F16, tag="xp")
            nc.tensor.transpose(e2q_ps, e2[:, q, :], eye16)
            nc.vector.tensor_copy(
                out=biasr[:, 128 * q : 128 * (q + 1)], in_=e2q_ps
            )

        # ---- c_i: load -> fp16 -> transposed [k, i] layout ----
        for m in range(MT):
            cit = xpool.tile([128, D], F32, tag="xload")
            nc.sync.dma_start(out=cit, in_=ci_ap[128 * m : 128 * (m + 1), :])
            c16 = epool.tile([128, D], F16, tag="c16")
            nc.vector.tensor_copy(out=c16, in_=cit)
            nc.scalar.dma_start_transpose(out=ciT[:, m], in_=c16)

        # ---- main matmuls + row softmax-exp accumulation ----
        for m in range(MT):
            for g in range(2):
                S_ps = psS.tile([128, JSH // 2], F32, tag="s")
                for kt in range(KT):
                    nc.tensor.matmul(
                        S_ps,
                        ciT[:, m, kt, :],
                        WT[:, kt, 4 * g : 4 * (g + 1), :],
                        start=(kt == 0),
                        stop=False,
                    )
                nc.tensor.matmul(
                    S_ps,
                    ones2,
                    biasr[:, 512 * g : 512 * (g + 1)],
                    start=False,
                    stop=True,
                )
                # local diag window (valid on p=0 cores for m<8, p=1 for m>=8)
                c0 = 128 * (m % 8)
                if c0 // 512 == g:
                    cg = c0 - 512 * g
                    junk = spool.tile([128, 128], F32, tag="junk")
                    nc.vector.tensor_mul(junk, S_ps[:, cg : cg + 128], eye32)
                    nc.vector.tensor_reduce(
                        out=Dc[:, m : m + 1], in_=junk, axis=AX.X, op=OP.add
                    )
                expj = dpool.tile([128, JSH // 2], F16, tag="expj")
                nc.scalar.activation(
                    out=expj,
                    in_=S_ps,
                    func=AF.Exp,
                    scale=float(INV_C),
                    accum_out=Zi[:, 2 * m + g : 2 * m + g + 1],
                )

        res = spool.tile([128, 3 * MT], F32, tag="res")
        nc.vector.tensor_copy(out=res[:, 0 : 2 * MT], in_=Zi)
        nc.vector.tensor_copy(out=res[:, 2 * MT : 3 * MT], in_=Dc)
        nc.sync.dma_start(out=out_ap, in_=res)


_NC_CACHE = {}


def build_nc():
    key = "nc_v3"
    if key in _NC_CACHE:
        return _NC_CACHE[key]
    nc = bacc.Bacc("TRN2", target_bir_lowering=False, debug=False)
    ci = nc.dram_tensor("ci", [ISH, D], F32, kind="ExternalInput").ap()
    cj = nc.dram_tensor("cj", [JSH, D], F32, kind="ExternalInput").ap()
    eye = nc.dram_tensor("eye", [128, 128], F32, kind="ExternalInput").ap()
    out = nc.dram_tensor("out", [128, 3 * MT], F32, kind="ExternalOutput").ap()
    with tile.TileContext(nc) as tc:
        build_kernel_body(tc, out, ci, cj, eye)
    nc.compile()
    _NC_CACHE[key] = nc
    return nc


def make_in_maps(c_i, c_j):
    eye = np.eye(128, dtype=np.float32)
    in_maps = []
    for c in range(NCORES):
        h, p = c // JQ, c % JQ
        sj = (ISH * h + JSH * p) % B
        in_maps.append(
            {
                "ci": np.ascontiguousarray(c_i[ISH * h : ISH * (h + 1)]),
                "cj": np.ascontiguousarray(c_j[sj : sj + JSH]),
                "eye": eye,
            }
        )
    return in_maps


def kernel(c_i, c_j, **kwargs):
    c_i = np.ascontiguousarray(np.asarray(c_i, dtype=np.float32))
    c_j = np.ascontiguousarray(np.asarray(c_j, dtype=np.float32))
    nc = build_nc()
    in_maps = make_in_maps(c_i, c_j)
    res = bass_utils.run_bass_kernel_spmd(
        nc, in_maps, core_ids=list(range(NCORES))
    )
    outs = [np.asarray(r["out"], dtype=np.float64) for r in res.results]
    loss = 0.0
    for h in range(IH):
        Zi_tot = sum(
            outs[JQ * h + p][:, 0 : 2 * MT : 2] + outs[JQ * h + p][:, 1 : 2 * MT : 2]
            for p in range(JQ)
        )
        lse = np.log(Zi_tot)
        diag = np.concatenate(
            [
                outs[JQ * h + 0][:, 2 * MT : 2 * MT + 8],
                outs[JQ * h + 1][:, 2 * MT + 8 : 2 * MT + 16],
            ],
            axis=1,
        )
        loss += (lse - diag * INV_C).sum()
    return np.float32(loss / B).reshape(())
